# revision 8
# baseline (speedup 1.0000x reference)
"""Trainium2 Bass kernel for nn_Attention (additive-attention scores + softmax).

Math: reference computes
    scores = (concat([hidden, enc], 1) @ W_att.T + b_att) @ w[0]
    attn   = softmax(scores)  over source_len
Since (x @ W.T) @ w == x @ (w @ W_att) and softmax is shift-invariant, the
hidden/b_att terms are constant shifts that cancel.  So:
    v2     = w[0] @ W_att[:, H:2H]          # [H]
    attn   = softmax(enc @ v2)
This turns a 137-GFLOP GEMM into two mat-vecs (memory-bound, ~80 MiB total).

Sharding (8 cores): enc row-sharded (1024 rows/core), W_att[:, H:] column-
sharded (256 cols/core, AllGather of the 256-wide v2 slices), scores
AllGathered so every core computes the full softmax.
"""

import sys

sys.path.insert(0, "/opt/trn_rl_repo")

import numpy as np

S, H = 8192, 2048
NCORES = 8
SS = S // NCORES      # 1024 enc rows per core
JS = H // NCORES      # 256 v2 columns per core
NT = SS // 128        # 8 enc tiles of [128, H] per core
KT = H // 128         # 16 k-tiles for the v2 matmul
FT = S // 128         # 64 scores per partition in the softmax


def _build(reps: int = 1):
    from concourse import bacc, mybir, tile, bass_isa
    import concourse.bass as bass

    f32 = mybir.dt.float32
    AT = mybir.AluOpType
    nc = bacc.Bacc(
        trn_type="TRN2", target_bir_lowering=False, debug=False, num_devices=NCORES
    )
    enc = nc.dram_tensor("enc", [SS, H], f32, kind="ExternalInput")
    w2 = nc.dram_tensor("w2", [H, JS], f32, kind="ExternalInput")
    wvec = nc.dram_tensor("wvec", [H], f32, kind="ExternalInput")
    out = nc.dram_tensor("out", [S], f32, kind="ExternalOutput")

    with tile.TileContext(nc) as tc:
        with (
            tc.tile_pool(name="dram", bufs=1, space="DRAM") as dram,
            tc.tile_pool(name="const", bufs=2) as const,
            tc.tile_pool(name="encp", bufs=6) as encp,
            tc.tile_pool(name="small", bufs=3) as small,
            tc.tile_pool(name="psum", bufs=1, space="PSUM") as psum,
        ):
            for _ in range(reps):
                cc_in_v2 = dram.tile([1, JS], f32)
                cc_out_v2 = dram.tile([NCORES, JS], f32, addr_space="Shared")
                cc_in_s = dram.tile([128, NT], f32)
                cc_out_s = dram.tile([NCORES * 128, NT], f32, addr_space="Shared")

                # Preload the exp activation table while DMAs stream.
                dummy = small.tile([1, 1], f32)
                nc.vector.memset(dummy, 0.0)
                nc.scalar.activation(
                    out=dummy, in_=dummy, func=mybir.ActivationFunctionType.Exp
                )

                # ---- v2_own = wvec @ w2  (k contracted on the PE) ----
                # row k = p*KT + t lives at partition p, slot t
                w_sb = const.tile([128, KT], f32)
                nc.sync.dma_start(out=w_sb, in_=wvec.ap().rearrange("(p t) -> p t", t=KT))
                w2_sb = const.tile([128, KT, JS], f32)
                w2r = w2.ap().rearrange("(p t) j -> p t j", t=KT)
                psum_v2 = psum.tile([1, JS], f32)
                CH = 4  # k-chunks per DMA so matmuls pipeline with the load
                for q in range(KT // CH):
                    nc.sync.dma_start(
                        out=w2_sb[:, q * CH : (q + 1) * CH, :],
                        in_=w2r[:, q * CH : (q + 1) * CH, :],
                    )
                    for t in range(q * CH, (q + 1) * CH):
                        nc.tensor.matmul(
                            psum_v2,
                            lhsT=w_sb[:, t : t + 1],
                            rhs=w2_sb[:, t, :],
                            start=(t == 0),
                            stop=(t == KT - 1),
                        )
                v2_own = small.tile([1, JS], f32)
                nc.scalar.copy(out=v2_own, in_=psum_v2)
                nc.scalar.dma_start(out=cc_in_v2, in_=v2_own)

                nc.gpsimd.collective_compute(
                    "AllGather",
                    AT.bypass,
                    replica_groups=[list(range(NCORES))],
                    ins=[cc_in_v2[:, :].opt()],
                    outs=[cc_out_v2[:, :].opt()],
                )

                # one SWDGE DMA replicates the gathered v2 row across all 128
                # partitions (stride-0 partition read from DRAM)
                v2rep = const.tile([128, H], f32)
                bcast_ap = bass.AP(
                    tensor=cc_out_v2.tensor,
                    offset=cc_out_v2.offset,
                    ap=[[0, 128], [1, H]],
                )
                nc.gpsimd.dma_start(out=v2rep, in_=bcast_ap)

                # ---- scores = enc @ v2 (fused mul+reduce on DVE) ----
                # enc row i = 8*p + n -> partition p, tile n; tiles DMA'd in
                # pairs (16 KiB contiguous per partition per transfer)
                encr = enc.ap().rearrange("(p n) d -> p n d", n=NT)
                scores_sb = const.tile([128, NT], f32)
                for g in range(NT // 2):
                    et2 = encp.tile([128, 2, H], f32, tag="et2", bufs=3)
                    nc.sync.dma_start(out=et2, in_=encr[:, 2 * g : 2 * g + 2, :])
                    for k in range(2):
                        n = 2 * g + k
                        if n == NT - 1:
                            # last tile on gpsimd+ACT to shorten the DVE span
                            nc.gpsimd.tensor_tensor(
                                et2[:, k, :], et2[:, k, :], v2rep, op=AT.mult
                            )
                            nc.scalar.activation(
                                out=et2[:, k, :],
                                in_=et2[:, k, :],
                                func=mybir.ActivationFunctionType.Copy,
                                accum_out=scores_sb[:, n : n + 1],
                            )
                        else:
                            nc.vector.affine_mul_reduce(
                                out=et2[:, k, :],
                                accum_out=scores_sb[:, n : n + 1],
                                in0=et2[:, k, :],
                                in1=v2rep,
                                scale=1.0,
                                bias=0.0,
                            )

                # ship the first half of the scores while tiles 4-7 compute
                nc.scalar.dma_start(
                    out=cc_in_s[:, 0 : NT // 2], in_=scores_sb[:, 0 : NT // 2]
                )
                nc.scalar.dma_start(
                    out=cc_in_s[:, NT // 2 :], in_=scores_sb[:, NT // 2 :]
                )
                nc.gpsimd.collective_compute(
                    "AllGather",
                    AT.bypass,
                    replica_groups=[list(range(NCORES))],
                    ins=[cc_in_s[:, :].opt()],
                    outs=[cc_out_s[:, :].opt()],
                )

                # ---- softmax over all S=8192 scores (global i = p*FT + f) ----
                sc = small.tile([128, FT], f32)
                nc.scalar.dma_start(
                    out=sc, in_=cc_out_s.rearrange("(p a) n -> p (a n)", a=FT // NT)
                )
                m = small.tile([128, 1], f32)
                nc.vector.reduce_max(out=m, in_=sc, axis=mybir.AxisListType.X)
                mb = small.tile([128, 1], f32)
                nc.gpsimd.partition_all_reduce(mb, m, 128, bass_isa.ReduceOp.max)
                negm = small.tile([128, 1], f32)
                nc.vector.tensor_scalar_mul(negm, mb, -1.0)
                e = small.tile([128, FT], f32)
                sume = small.tile([128, 1], f32)
                nc.scalar.activation(
                    out=e,
                    in_=sc,
                    func=mybir.ActivationFunctionType.Exp,
                    bias=negm,
                    scale=1.0,
                    accum_out=sume,
                )
                sumb = small.tile([128, 1], f32)
                nc.gpsimd.partition_all_reduce(sumb, sume, 128, bass_isa.ReduceOp.add)
                rinv = small.tile([128, 1], f32)
                nc.vector.reciprocal(rinv, sumb)
                attn = small.tile([128, FT], f32)
                nc.vector.tensor_scalar_mul(attn, e, rinv)
                nc.scalar.dma_start(
                    out=out.ap().rearrange("(p f) -> p f", f=FT), in_=attn
                )
    nc.finalize()
    return nc


_NC_CACHE: dict = {}


def get_nc(reps: int = 1):
    if reps not in _NC_CACHE:
        _NC_CACHE[reps] = _build(reps)
    return _NC_CACHE[reps]


def make_in_maps(encoder_outputs, hidden, W_att, b_att, w):
    enc = np.ascontiguousarray(np.asarray(encoder_outputs)[:, 0, :], dtype=np.float32)
    wv = np.ascontiguousarray(np.asarray(w)[0], dtype=np.float32)
    W = np.asarray(W_att)
    in_maps = []
    for c in range(NCORES):
        in_maps.append(
            {
                "enc": np.ascontiguousarray(enc[c * SS : (c + 1) * SS]),
                "w2": np.ascontiguousarray(
                    W[:, H + c * JS : H + (c + 1) * JS], dtype=np.float32
                ),
                "wvec": wv,
            }
        )
    return in_maps


def kernel(encoder_outputs, hidden, W_att, b_att, w):
    from concourse import bass_utils

    nc = get_nc(reps=1)
    in_maps = make_in_maps(encoder_outputs, hidden, W_att, b_att, w)
    res = bass_utils.run_bass_kernel_spmd(
        nc, in_maps, core_ids=list(range(NCORES)), trace=False
    )
    attn = np.asarray(res.results[0]["out"], dtype=np.float32)
    return attn[None, None, :]
